# revision 20
# baseline (speedup 1.0000x reference)
"""Trainium2 Bass kernel for nn_AttentionLayer (sparse_attention).

Math (per batch b, history l):
    info = [q, k, q-k, q*k] @ W1 + b1 ; @ W2 + b2 ; sigmoid ; @ Wf + bf
    score = softmax(where(mask, -inf, logit), axis=l)
    out   = sum_l score * v

Host-side algebra (exact up to fp assoc):
  - No nonlinearity between W1/W2  =>  h2 = k@P + (q*k)@Q + r_b
        P = (W1b-W1c)@W2, Q = W1d@W2, r_b = q_b@(W1a+W1c)@W2 + b1@W2 + b2
  - Fold q into per-batch weights: h2 = k @ V_b + r_b,  V_b = P + diag(q_b) Q
  - Fold r_b into k: solve s_b @ V_b = r_b (least-norm), ship k + s_b
  - sigmoid(x)@Wf = tanh(x/2)@(Wf/2) + const; const cancels in softmax
  - MASK COMPACTION: masked tokens (exp(-inf)=0) are dropped on host; each
    batch's <=126 unmasked tokens are packed into 128 slots (pads: k=0 ->
    logit 0, madd=-30, v=0). Halves k/v traffic and all device compute.
Device layout: token-major 2-stream columns (batch-pair r -> 128 cols,
partitions 0:64 = stream-A E-dims, 64:128 = stream-B). One block-diagonal
[128,80] matmul per pair -> h2 [80,128] (A h2 parts 0:40, B 40:80); tanh
(scale .5) -> t bf16; wf matmuls [80,2] write logits into 4 PSUM partition
strips (32s, 32s+1) at N=512; ACT-copy evacuates [98,512] to bf16 staging;
8 strided DMAs per quarter land logits batch-major [128,128]; softmax + p@v
on DVE (exp w/ accum z on ACT; mult + 2 folds + reduce + scale).
"""

import sys

sys.path.insert(0, "/opt/trn_rl_repo")

import numpy as np
import ml_dtypes

import concourse.bass as bass
import concourse.bacc as bacc
import concourse.tile as tile
import concourse.mybir as mybir
from concourse.bass_utils import run_bass_kernel_spmd

N_CORES = 8
B_FULL = 4096
B = B_FULL // N_CORES  # 512 batches per core
E = 64
H = 40
LP = 128               # compacted history slots per batch
NPAIR = B // 2         # 256 batch pairs per core
NGRP = 16              # pairs per group (one h2 psum tile)
NSLAB = 8              # kx/vw DMA slabs (32 pairs each)

BF16 = mybir.dt.bfloat16
FP8 = mybir.dt.float8e4
F32 = mybir.dt.float32
nbf16 = ml_dtypes.bfloat16
nfp8 = ml_dtypes.float8_e4m3fn


def build_nc():
    nc = bacc.Bacc()

    kx_d = nc.declare_dram_parameter("kx", [128, NPAIR * LP], FP8, isOutput=False)
    vw_d = nc.declare_dram_parameter("vw", [128, NPAIR * 80], FP8, isOutput=False)
    wf_d = nc.declare_dram_parameter("wf2", [80, 2], BF16, isOutput=False)
    vt_d = nc.declare_dram_parameter("vt", [B, E * LP], BF16, isOutput=False)
    madd_d = nc.declare_dram_parameter("madd", [B, LP], BF16, isOutput=False)
    out_d = nc.declare_dram_parameter("out", [B, E], F32, isOutput=True)

    Tanh = mybir.ActivationFunctionType.Tanh
    Exp = mybir.ActivationFunctionType.Exp
    Copy = mybir.ActivationFunctionType.Copy
    Alu = mybir.AluOpType
    X = mybir.AxisListType.X

    SLABC = 2 * NGRP * LP   # kx cols per slab (4096)
    SLABW = 2 * NGRP * 80   # vw cols per slab (2560)

    from contextlib import ExitStack

    with tile.TileContext(nc) as tc, ExitStack() as ctx:
        const = ctx.enter_context(tc.tile_pool(name="const", bufs=1))
        kxp = ctx.enter_context(tc.tile_pool(name="kxp", bufs=1))
        vwp = ctx.enter_context(tc.tile_pool(name="vwp", bufs=1))
        h2p = ctx.enter_context(tc.tile_pool(name="h2p", bufs=2, space="PSUM"))
        lgp = ctx.enter_context(tc.tile_pool(name="lgp", bufs=2, space="PSUM"))
        tp = ctx.enter_context(tc.tile_pool(name="tp", bufs=2))
        stp = ctx.enter_context(tc.tile_pool(name="stp", bufs=1))
        lmp = ctx.enter_context(tc.tile_pool(name="lmp", bufs=2))
        vtp = ctx.enter_context(tc.tile_pool(name="vtp", bufs=2))
        mp = ctx.enter_context(tc.tile_pool(name="mp", bufs=2))
        bp = ctx.enter_context(tc.tile_pool(name="bp", bufs=1))

        wf_t = const.tile([80, 2], BF16, tag="wf")
        nc.sync.dma_start(wf_t[:], wf_d[:])

        kx_t = {}
        vw_t = {}

        # variable slabs (in groups of 8 pairs): small first slabs so the
        # first matmul starts after ~256KB of DMA instead of 1.6MB
        SLAB_GROUPS = [1, 1, 2, 4, 4, 4, 4, 4, 4, 4]
        SLAB_G0 = np.cumsum([0] + SLAB_GROUPS).tolist()

        def load_slab(s):
            ng = SLAB_GROUPS[s]
            g0 = SLAB_G0[s]
            kt = kxp.tile([128, ng * 8 * LP], FP8, tag=f"kx{s}", name=f"kx{s}")
            nc.sync.dma_start(kt[:], kx_d[:, g0 * 8 * LP:(g0 + ng) * 8 * LP])
            kx_t[s] = kt
            wt = vwp.tile([128, ng * 8 * 80], FP8, tag=f"vw{s}", name=f"vw{s}")
            nc.sync.dma_start(wt[:], vw_d[:, g0 * 8 * 80:(g0 + ng) * 8 * 80])
            vw_t[s] = wt

        qdat = {}

        def load_quarter(qq, chunk):
            # same (sync) ring as the kx/vw slabs so arrival order matches
            # consumption order, but in 512KB chunks interleaved between
            # slab loads so the slab stream never stalls behind a 2MB blob
            if chunk == 0:
                vt_t = vtp.tile([128, E * LP], BF16, tag="vt", name=f"vt{qq}")
                md_t = mp.tile([128, LP], BF16, tag="md", name=f"md{qq}")
                nc.gpsimd.dma_start(md_t[:], madd_d[qq * 128:(qq + 1) * 128, :])
                qdat[qq] = (vt_t, md_t)
            vt_t = qdat[qq][0]
            c0 = chunk * (E * LP // 4)
            c1 = (chunk + 1) * (E * LP // 4)
            nc.gpsimd.dma_start(vt_t[:, c0:c1], vt_d[qq * 128:(qq + 1) * 128, c0:c1])

        def phase_b(qq, lm_t):
            vt_t, md_t = qdat.pop(qq)
            ladj = bp.tile([128, LP], F32, tag="ladj", name=f"ladj{qq}")
            nc.vector.tensor_tensor(ladj[:], lm_t[:], md_t[:], Alu.add)
            p_t = bp.tile([128, LP], BF16, tag="p", name=f"p{qq}")
            z_t = bp.tile([128, 1], F32, tag="z", name=f"z{qq}")
            nc.scalar.activation(p_t[:], ladj[:], Exp, accum_out=z_t[:])

            w1 = bp.tile([128, E * LP], BF16, tag="w1", name=f"w1{qq}")
            p_b = p_t[:].rearrange("p (o l) -> p o l", o=1).broadcast_to([128, E, LP])
            nc.vector.tensor_tensor(
                w1[:].rearrange("p (e l) -> p e l", e=E),
                vt_t[:].rearrange("p (e l) -> p e l", e=E),
                p_b, Alu.mult,
            )
            w2 = bp.tile([128, E * LP // 2], BF16, tag="w2", name=f"w2{qq}")
            w1v = w1[:].rearrange("p (e l) -> p e l", e=E)
            nc.vector.tensor_tensor(
                w2[:].rearrange("p (e l) -> p e l", e=E),
                w1v[:, :, 0:LP // 2], w1v[:, :, LP // 2:LP], Alu.add,
            )
            w3 = bp.tile([128, E * LP // 4], BF16, tag="w3", name=f"w3{qq}")
            w2v = w2[:].rearrange("p (e l) -> p e l", e=E)
            nc.vector.tensor_tensor(
                w3[:].rearrange("p (e l) -> p e l", e=E),
                w2v[:, :, 0:LP // 4], w2v[:, :, LP // 4:LP // 2], Alu.add,
            )
            acc = bp.tile([128, E], F32, tag="acc", name=f"acc{qq}")
            nc.vector.tensor_reduce(
                acc[:], w3[:].rearrange("p (e l) -> p e l", e=E), axis=X, op=Alu.add)
            rz = bp.tile([128, 1], F32, tag="rz", name=f"rz{qq}")
            nc.vector.reciprocal(rz[:], z_t[:])
            o_t = bp.tile([128, E], F32, tag="o", name=f"o{qq}")
            nc.vector.tensor_scalar_mul(o_t[:], acc[:], rz[:])
            nc.gpsimd.dma_start(out_d[qq * 128:(qq + 1) * 128, :], o_t[:])

        load_slab(0)
        st_t = None
        lg_t = None
        GP = 8  # pairs per h2 group ([80, 1024] f32 = 2 psum banks)
        slab_of_group = []
        for si, ng in enumerate(SLAB_GROUPS):
            slab_of_group += [si] * ng
        for g in range(32):
            s = slab_of_group[g]
            if g == SLAB_G0[s] and s + 1 < len(SLAB_GROUPS):
                load_slab(s + 1)
            if 4 <= g < 8:
                load_quarter(0, g - 4)
            elif g >= 8 and 2 <= g % 8 < 6:
                load_quarter(g // 8, g % 8 - 2)

            kxs, vws = kx_t[s], vw_t[s]
            h2_t = h2p.tile([80, GP * LP], F32, tag="h2", name=f"h2_{g}")
            for pp in range(GP):
                rr = (g - SLAB_G0[s]) * GP + pp  # pair within slab
                nc.tensor.matmul(
                    h2_t[0:80, pp * LP:(pp + 1) * LP],
                    vws[:, rr * 80:rr * 80 + 80],
                    kxs[:, rr * LP:(rr + 1) * LP],
                    start=True, stop=True,
                )
            t_t = tp.tile([80, GP * LP], BF16, tag="t", name=f"t_{g}")
            nc.scalar.activation(t_t[:], h2_t[:], Tanh, scale=0.5)

            if g % 2 == 0:
                lg_t = lgp.tile([98, 512], F32, tag="lg", name=f"lg_{g // 2}")
            for j in range(2):
                ss = 2 * (g % 2) + j
                nc.tensor.matmul(
                    lg_t[32 * ss:32 * ss + 2, 0:512],
                    wf_t[:], t_t[:, 512 * j:512 * (j + 1)],
                    start=True, stop=True, tile_position=(0, 32 * ss),
                )
            if g % 2 == 1:
                qq, gq = g // 8, (g // 2) % 4
                if gq == 0:
                    st_t = stp.tile([98, 4 * 512], BF16, tag="st", name=f"st{qq}")
                nc.scalar.activation(
                    st_t[:, 512 * gq:512 * (gq + 1)], lg_t[:], Copy)

                if gq == 3:
                    lm_t = lmp.tile([128, LP], BF16, tag="lm", name=f"lm{qq}")
                    # rows {32s, 32s+1} unfold to batch-major rows
                    # 32s..32s+32 in one DMA (row-major both sides);
                    # last quarter rides the idle scalar HWDGE ring to cut
                    # the end-of-kernel staging latency
                    for ss in range(4):
                        nc.gpsimd.dma_start(
                            lm_t[32 * ss:32 * ss + 32, :],
                            st_t[32 * ss:32 * ss + 2, :])
                    phase_b(qq, lm_t)

    if not nc.is_finalized():
        nc.finalize()
    return nc


def host_prep(q, k, v, mask, W1, b1, W2, b2, Wf, bf):
    """Fold weights per batch, compact masked tokens, build device layouts."""
    q2 = q[:, 0, :].astype(np.float32)
    W1 = W1.astype(np.float32); W2 = W2.astype(np.float32)
    P = (W1[64:128] - W1[128:192]) @ W2                     # [64,40]
    Q = W1[192:256] @ W2                                    # [64,40]
    A2 = (W1[0:64] + W1[128:192]) @ W2
    c0 = b1.astype(np.float32) @ W2 + b2.astype(np.float32)
    r = q2 @ A2 + c0                                        # [Bf,40]
    V = P[None] + q2[:, :, None] * Q[None]                  # [Bf,64,40]
    G = np.einsum('beh,bei->bhi', V, V)
    y = np.linalg.solve(G, r[:, :, None])
    s = np.einsum('beh,bhx->be', V, y)                      # [Bf,64]

    m = mask[:, :, 0]
    order = np.argsort(m, axis=1, kind='stable')[:, :LP]
    nvalid = (~m).sum(1)
    assert nvalid.max() <= LP, f"batch with {nvalid.max()} unmasked tokens"
    validc = np.arange(LP)[None, :] < nvalid[:, None]       # [Bf,LP]
    kc = np.take_along_axis(k.astype(np.float32), order[:, :, None], 1)
    vc = np.take_along_axis(v.astype(np.float32), order[:, :, None], 1)
    kc = np.where(validc[..., None], kc + s[:, None, :], 0.0)
    vc = np.where(validc[..., None], vc, 0.0)
    maddf = np.where(validc, np.float32(0.0), np.float32(-30.0)).astype(nbf16)

    # core-local batch <-> (pair r, stream sig) map
    b = np.arange(B)
    qq = b // 128; t = b % 128
    s2s = t // 16; s_ = s2s // 2; sig = s2s % 2
    g_ = (t % 16) // 4; cb = t % 4
    r_ = 64 * qq + 16 * g_ + 4 * s_ + cb
    A_idx = np.empty(NPAIR, np.int64); B_idx = np.empty(NPAIR, np.int64)
    A_idx[r_[sig == 0]] = b[sig == 0]
    B_idx[r_[sig == 1]] = b[sig == 1]

    in_maps = []
    for c in range(N_CORES):
        sl = slice(c * B, (c + 1) * B)
        kcc, Vc = kc[sl], V[sl]
        kx = np.empty((128, NPAIR * LP), np.float32)
        kx[0:64] = kcc[A_idx].transpose(2, 0, 1).reshape(64, -1)
        kx[64:128] = kcc[B_idx].transpose(2, 0, 1).reshape(64, -1)
        vw3 = np.zeros((NPAIR, 128, 80), np.float32)
        vw3[:, 0:64, 0:40] = Vc[A_idx]
        vw3[:, 64:128, 40:80] = Vc[B_idx]
        vw = vw3.transpose(1, 0, 2).reshape(128, NPAIR * 80)
        vt = np.ascontiguousarray(vc[sl].transpose(0, 2, 1)).reshape(B, E * LP)
        wf2 = np.zeros((80, 2), np.float32)
        wf2[0:40, 0] = 0.5 * Wf[:, 0]
        wf2[40:80, 1] = 0.5 * Wf[:, 0]
        in_maps.append({
            "kx": np.ascontiguousarray(kx).astype(nfp8),
            "vw": np.ascontiguousarray(vw).astype(nfp8),
            "wf2": wf2.astype(nbf16),
            "vt": vt.astype(nbf16),
            "madd": np.ascontiguousarray(maddf[sl]),
        })
    return in_maps


_CACHE = {}


def run_on_device(in_maps, trace=False):
    if "nc" not in _CACHE:
        _CACHE["nc"] = build_nc()
    nc = _CACHE["nc"]
    res = run_bass_kernel_spmd(nc, in_maps, core_ids=list(range(N_CORES)),
                               trace=trace)
    return res


def kernel(q, k, v, mask, W1, b1, W2, b2, Wf, bf):
    in_maps = host_prep(q, k, v, mask, W1, b1, W2, b2, Wf, bf)
    res = run_on_device(in_maps)
    out = np.concatenate([res.results[c]["out"] for c in range(N_CORES)], axis=0)
    return out.astype(np.float32)


# revision 21
# speedup vs baseline: 1.1320x; 1.1320x over previous
"""Trainium2 Bass kernel for nn_AttentionLayer (sparse_attention).

Math (per batch b, history l):
    info = [q, k, q-k, q*k] @ W1 + b1 ; @ W2 + b2 ; sigmoid ; @ Wf + bf
    score = softmax(where(mask, -inf, logit), axis=l)
    out   = sum_l score * v

Host-side algebra (exact up to fp assoc):
  - No nonlinearity between W1/W2  =>  h2 = k@P + (q*k)@Q + r_b
        P = (W1b-W1c)@W2, Q = W1d@W2, r_b = q_b@(W1a+W1c)@W2 + b1@W2 + b2
  - Fold q into per-batch weights: h2 = k @ V_b + r_b,  V_b = P + diag(q_b) Q
  - Fold r_b into k: solve s_b @ V_b = r_b (least-norm), ship k + s_b
  - sigmoid(x)@Wf = tanh(x/2)@(Wf/2) + const; const cancels in softmax
  - MASK COMPACTION: masked tokens (exp(-inf)=0) are dropped on host; each
    batch's <=126 unmasked tokens are packed into 128 slots (pads: k=0 ->
    logit 0, madd=-30, v=0). Halves k/v traffic and all device compute.
Device layout: token-major 2-stream columns (batch-pair r -> 128 cols,
partitions 0:64 = stream-A E-dims, 64:128 = stream-B). One block-diagonal
[128,80] matmul per pair -> h2 [80,128] (A h2 parts 0:40, B 40:80); tanh
(scale .5) -> t bf16; wf matmuls [80,2] write logits into 4 PSUM partition
strips (32s, 32s+1) at N=512; ACT-copy evacuates [98,512] to bf16 staging;
8 strided DMAs per quarter land logits batch-major [128,128]; softmax + p@v
on DVE (exp w/ accum z on ACT; mult + 2 folds + reduce + scale).
"""

import sys

sys.path.insert(0, "/opt/trn_rl_repo")

import numpy as np
import ml_dtypes

import concourse.bass as bass
import concourse.bacc as bacc
import concourse.tile as tile
import concourse.mybir as mybir
from concourse.bass_utils import run_bass_kernel_spmd

N_CORES = 8
B_FULL = 4096
B = B_FULL // N_CORES  # 512 batches per core
E = 64
H = 40
LP = 128               # compacted history slots per batch
NPAIR = B // 2         # 256 batch pairs per core
NGRP = 16              # pairs per group (one h2 psum tile)
NSLAB = 8              # kx/vw DMA slabs (32 pairs each)

BF16 = mybir.dt.bfloat16
FP8 = mybir.dt.float8e4
F32 = mybir.dt.float32
nbf16 = ml_dtypes.bfloat16
nfp8 = ml_dtypes.float8_e4m3fn


def build_nc():
    nc = bacc.Bacc()

    kx_d = nc.declare_dram_parameter("kx", [128, NPAIR * LP], FP8, isOutput=False)
    vw_d = nc.declare_dram_parameter("vw", [128, NPAIR * 80], FP8, isOutput=False)
    wf_d = nc.declare_dram_parameter("wf2", [80, 2], BF16, isOutput=False)
    vt_d = nc.declare_dram_parameter("vt", [B, E * LP], BF16, isOutput=False)
    madd_d = nc.declare_dram_parameter("madd", [B, LP], BF16, isOutput=False)
    out_d = nc.declare_dram_parameter("out", [B, E], F32, isOutput=True)

    Tanh = mybir.ActivationFunctionType.Tanh
    Exp = mybir.ActivationFunctionType.Exp
    Copy = mybir.ActivationFunctionType.Copy
    Alu = mybir.AluOpType
    X = mybir.AxisListType.X

    SLABC = 2 * NGRP * LP   # kx cols per slab (4096)
    SLABW = 2 * NGRP * 80   # vw cols per slab (2560)

    from contextlib import ExitStack

    with tile.TileContext(nc) as tc, ExitStack() as ctx:
        const = ctx.enter_context(tc.tile_pool(name="const", bufs=1))
        kxp = ctx.enter_context(tc.tile_pool(name="kxp", bufs=1))
        vwp = ctx.enter_context(tc.tile_pool(name="vwp", bufs=1))
        h2p = ctx.enter_context(tc.tile_pool(name="h2p", bufs=2, space="PSUM"))
        lgp = ctx.enter_context(tc.tile_pool(name="lgp", bufs=2, space="PSUM"))
        tp = ctx.enter_context(tc.tile_pool(name="tp", bufs=2))
        stp = ctx.enter_context(tc.tile_pool(name="stp", bufs=2))
        lmp = ctx.enter_context(tc.tile_pool(name="lmp", bufs=2))
        vtp = ctx.enter_context(tc.tile_pool(name="vtp", bufs=2))
        mp = ctx.enter_context(tc.tile_pool(name="mp", bufs=2))
        bp = ctx.enter_context(tc.tile_pool(name="bp", bufs=1))

        wf_t = const.tile([80, 2], BF16, tag="wf")
        nc.sync.dma_start(wf_t[:], wf_d[:])

        kx_t = {}
        vw_t = {}

        # variable slabs (in groups of 8 pairs): small first slabs so the
        # first matmul starts after ~256KB of DMA instead of 1.6MB
        SLAB_GROUPS = [1, 1, 2, 4, 4, 4, 4, 4, 4, 4]
        SLAB_G0 = np.cumsum([0] + SLAB_GROUPS).tolist()

        def load_slab(s):
            ng = SLAB_GROUPS[s]
            g0 = SLAB_G0[s]
            kt = kxp.tile([128, ng * 8 * LP], FP8, tag=f"kx{s}", name=f"kx{s}")
            nc.sync.dma_start(kt[:], kx_d[:, g0 * 8 * LP:(g0 + ng) * 8 * LP])
            kx_t[s] = kt
            wt = vwp.tile([128, ng * 8 * 80], FP8, tag=f"vw{s}", name=f"vw{s}")
            nc.sync.dma_start(wt[:], vw_d[:, g0 * 8 * 80:(g0 + ng) * 8 * 80])
            vw_t[s] = wt

        qdat = {}

        def load_quarter(qq, chunk):
            # same (sync) ring as the kx/vw slabs so arrival order matches
            # consumption order, but in 512KB chunks interleaved between
            # slab loads so the slab stream never stalls behind a 2MB blob
            if chunk == 0:
                vt_t = vtp.tile([128, E * LP], BF16, tag="vt", name=f"vt{qq}")
                md_t = mp.tile([128, LP], BF16, tag="md", name=f"md{qq}")
                nc.gpsimd.dma_start(md_t[:], madd_d[qq * 128:(qq + 1) * 128, :])
                qdat[qq] = (vt_t, md_t)
            vt_t = qdat[qq][0]
            c0 = chunk * (E * LP // 4)
            c1 = (chunk + 1) * (E * LP // 4)
            nc.gpsimd.dma_start(vt_t[:, c0:c1], vt_d[qq * 128:(qq + 1) * 128, c0:c1])

        def phase_b(qq, lm_t):
            vt_t, md_t = qdat.pop(qq)
            ladj = bp.tile([128, LP], F32, tag="ladj", name=f"ladj{qq}")
            nc.vector.tensor_tensor(ladj[:], lm_t[:], md_t[:], Alu.add)
            p_t = bp.tile([128, LP], BF16, tag="p", name=f"p{qq}")
            z_t = bp.tile([128, 1], F32, tag="z", name=f"z{qq}")
            nc.scalar.activation(p_t[:], ladj[:], Exp, accum_out=z_t[:])

            w1 = bp.tile([128, E * LP], BF16, tag="w1", name=f"w1{qq}")
            p_b = p_t[:].rearrange("p (o l) -> p o l", o=1).broadcast_to([128, E, LP])
            nc.vector.tensor_tensor(
                w1[:].rearrange("p (e l) -> p e l", e=E),
                vt_t[:].rearrange("p (e l) -> p e l", e=E),
                p_b, Alu.mult,
            )
            w2 = bp.tile([128, E * LP // 2], BF16, tag="w2", name=f"w2{qq}")
            w1v = w1[:].rearrange("p (e l) -> p e l", e=E)
            nc.vector.tensor_tensor(
                w2[:].rearrange("p (e l) -> p e l", e=E),
                w1v[:, :, 0:LP // 2], w1v[:, :, LP // 2:LP], Alu.add,
            )
            w3 = bp.tile([128, E * LP // 4], BF16, tag="w3", name=f"w3{qq}")
            w2v = w2[:].rearrange("p (e l) -> p e l", e=E)
            nc.vector.tensor_tensor(
                w3[:].rearrange("p (e l) -> p e l", e=E),
                w2v[:, :, 0:LP // 4], w2v[:, :, LP // 4:LP // 2], Alu.add,
            )
            acc = bp.tile([128, E], F32, tag="acc", name=f"acc{qq}")
            nc.vector.tensor_reduce(
                acc[:], w3[:].rearrange("p (e l) -> p e l", e=E), axis=X, op=Alu.add)
            rz = bp.tile([128, 1], F32, tag="rz", name=f"rz{qq}")
            nc.vector.reciprocal(rz[:], z_t[:])
            o_t = bp.tile([128, E], F32, tag="o", name=f"o{qq}")
            nc.vector.tensor_scalar_mul(o_t[:], acc[:], rz[:])
            nc.gpsimd.dma_start(out_d[qq * 128:(qq + 1) * 128, :], o_t[:])

        load_slab(0)
        st_t = None
        lg_t = None
        GP = 8  # pairs per h2 group ([80, 1024] f32 = 2 psum banks)
        slab_of_group = []
        for si, ng in enumerate(SLAB_GROUPS):
            slab_of_group += [si] * ng
        for g in range(32):
            s = slab_of_group[g]
            if g == SLAB_G0[s] and s + 1 < len(SLAB_GROUPS):
                load_slab(s + 1)
            if 4 <= g < 8:
                load_quarter(0, g - 4)
            elif g >= 8 and 2 <= g % 8 < 6:
                load_quarter(g // 8, g % 8 - 2)

            kxs, vws = kx_t[s], vw_t[s]
            h2_t = h2p.tile([80, GP * LP], F32, tag="h2", name=f"h2_{g}")
            for pp in range(GP):
                rr = (g - SLAB_G0[s]) * GP + pp  # pair within slab
                nc.tensor.matmul(
                    h2_t[0:80, pp * LP:(pp + 1) * LP],
                    vws[:, rr * 80:rr * 80 + 80],
                    kxs[:, rr * LP:(rr + 1) * LP],
                    start=True, stop=True,
                )
            t_t = tp.tile([80, GP * LP], BF16, tag="t", name=f"t_{g}")
            nc.scalar.activation(t_t[:], h2_t[:], Tanh, scale=0.5)

            if g % 2 == 0:
                lg_t = lgp.tile([98, 512], F32, tag="lg", name=f"lg_{g // 2}")
            for j in range(2):
                ss = 2 * (g % 2) + j
                nc.tensor.matmul(
                    lg_t[32 * ss:32 * ss + 2, 0:512],
                    wf_t[:], t_t[:, 512 * j:512 * (j + 1)],
                    start=True, stop=True, tile_position=(0, 32 * ss),
                )
            if g % 2 == 1:
                qq, gq = g // 8, (g // 2) % 4
                if gq == 0:
                    st_t = stp.tile([98, 4 * 512], BF16, tag="st", name=f"st{qq}")
                nc.scalar.activation(
                    st_t[:, 512 * gq:512 * (gq + 1)], lg_t[:], Copy)

                if gq == 3:
                    lm_t = lmp.tile([128, LP], BF16, tag="lm", name=f"lm{qq}")
                    # rows {32s, 32s+1} unfold to batch-major rows
                    # 32s..32s+32 in one DMA (row-major both sides);
                    # last quarter rides the idle scalar HWDGE ring to cut
                    # the end-of-kernel staging latency
                    deng = nc.scalar if qq == 3 else nc.gpsimd
                    for ss in range(4):
                        deng.dma_start(
                            lm_t[32 * ss:32 * ss + 32, :],
                            st_t[32 * ss:32 * ss + 2, :])
                    phase_b(qq, lm_t)

    if not nc.is_finalized():
        nc.finalize()
    return nc


def host_prep(q, k, v, mask, W1, b1, W2, b2, Wf, bf):
    """Fold weights per batch, compact masked tokens, build device layouts."""
    q2 = q[:, 0, :].astype(np.float32)
    W1 = W1.astype(np.float32); W2 = W2.astype(np.float32)
    P = (W1[64:128] - W1[128:192]) @ W2                     # [64,40]
    Q = W1[192:256] @ W2                                    # [64,40]
    A2 = (W1[0:64] + W1[128:192]) @ W2
    c0 = b1.astype(np.float32) @ W2 + b2.astype(np.float32)
    r = q2 @ A2 + c0                                        # [Bf,40]
    V = P[None] + q2[:, :, None] * Q[None]                  # [Bf,64,40]
    G = np.einsum('beh,bei->bhi', V, V)
    y = np.linalg.solve(G, r[:, :, None])
    s = np.einsum('beh,bhx->be', V, y)                      # [Bf,64]

    m = mask[:, :, 0]
    order = np.argsort(m, axis=1, kind='stable')[:, :LP]
    nvalid = (~m).sum(1)
    assert nvalid.max() <= LP, f"batch with {nvalid.max()} unmasked tokens"
    validc = np.arange(LP)[None, :] < nvalid[:, None]       # [Bf,LP]
    kc = np.take_along_axis(k.astype(np.float32), order[:, :, None], 1)
    vc = np.take_along_axis(v.astype(np.float32), order[:, :, None], 1)
    kc = np.where(validc[..., None], kc + s[:, None, :], 0.0)
    vc = np.where(validc[..., None], vc, 0.0)
    maddf = np.where(validc, np.float32(0.0), np.float32(-30.0)).astype(nbf16)

    # core-local batch <-> (pair r, stream sig) map
    b = np.arange(B)
    qq = b // 128; t = b % 128
    s2s = t // 16; s_ = s2s // 2; sig = s2s % 2
    g_ = (t % 16) // 4; cb = t % 4
    r_ = 64 * qq + 16 * g_ + 4 * s_ + cb
    A_idx = np.empty(NPAIR, np.int64); B_idx = np.empty(NPAIR, np.int64)
    A_idx[r_[sig == 0]] = b[sig == 0]
    B_idx[r_[sig == 1]] = b[sig == 1]

    in_maps = []
    for c in range(N_CORES):
        sl = slice(c * B, (c + 1) * B)
        kcc, Vc = kc[sl], V[sl]
        kx = np.empty((128, NPAIR * LP), np.float32)
        kx[0:64] = kcc[A_idx].transpose(2, 0, 1).reshape(64, -1)
        kx[64:128] = kcc[B_idx].transpose(2, 0, 1).reshape(64, -1)
        vw3 = np.zeros((NPAIR, 128, 80), np.float32)
        vw3[:, 0:64, 0:40] = Vc[A_idx]
        vw3[:, 64:128, 40:80] = Vc[B_idx]
        vw = vw3.transpose(1, 0, 2).reshape(128, NPAIR * 80)
        vt = np.ascontiguousarray(vc[sl].transpose(0, 2, 1)).reshape(B, E * LP)
        wf2 = np.zeros((80, 2), np.float32)
        wf2[0:40, 0] = 0.5 * Wf[:, 0]
        wf2[40:80, 1] = 0.5 * Wf[:, 0]
        in_maps.append({
            "kx": np.ascontiguousarray(kx).astype(nfp8),
            "vw": np.ascontiguousarray(vw).astype(nfp8),
            "wf2": wf2.astype(nbf16),
            "vt": vt.astype(nbf16),
            "madd": np.ascontiguousarray(maddf[sl]),
        })
    return in_maps


_CACHE = {}


def run_on_device(in_maps, trace=False):
    if "nc" not in _CACHE:
        _CACHE["nc"] = build_nc()
    nc = _CACHE["nc"]
    res = run_bass_kernel_spmd(nc, in_maps, core_ids=list(range(N_CORES)),
                               trace=trace)
    return res


def kernel(q, k, v, mask, W1, b1, W2, b2, Wf, bf):
    in_maps = host_prep(q, k, v, mask, W1, b1, W2, b2, Wf, bf)
    res = run_on_device(in_maps)
    out = np.concatenate([res.results[c]["out"] for c in range(N_CORES)], axis=0)
    return out.astype(np.float32)


# revision 22
# speedup vs baseline: 1.1523x; 1.0179x over previous
"""Trainium2 Bass kernel for nn_AttentionLayer (sparse_attention).

Math (per batch b, history l):
    info = [q, k, q-k, q*k] @ W1 + b1 ; @ W2 + b2 ; sigmoid ; @ Wf + bf
    score = softmax(where(mask, -inf, logit), axis=l)
    out   = sum_l score * v

Host-side algebra (exact up to fp assoc):
  - No nonlinearity between W1/W2  =>  h2 = k@P + (q*k)@Q + r_b
        P = (W1b-W1c)@W2, Q = W1d@W2, r_b = q_b@(W1a+W1c)@W2 + b1@W2 + b2
  - Fold q into per-batch weights: h2 = k @ V_b + r_b,  V_b = P + diag(q_b) Q
  - Fold r_b into k: solve s_b @ V_b = r_b (least-norm), ship k + s_b
  - sigmoid(x)@Wf = tanh(x/2)@(Wf/2) + const; const cancels in softmax
  - MASK COMPACTION: masked tokens (exp(-inf)=0) are dropped on host; each
    batch's <=126 unmasked tokens are packed into 128 slots (pads: k=0 ->
    logit 0, madd=-30, v=0). Halves k/v traffic and all device compute.
Device layout: token-major 2-stream columns (batch-pair r -> 128 cols,
partitions 0:64 = stream-A E-dims, 64:128 = stream-B). One block-diagonal
[128,80] matmul per pair -> h2 [80,128] (A h2 parts 0:40, B 40:80); tanh
(scale .5) -> t bf16; wf matmuls [80,2] write logits into 4 PSUM partition
strips (32s, 32s+1) at N=512; ACT-copy evacuates [98,512] to bf16 staging;
8 strided DMAs per quarter land logits batch-major [128,128]; softmax + p@v
on DVE (exp w/ accum z on ACT; mult + 2 folds + reduce + scale).
"""

import sys

sys.path.insert(0, "/opt/trn_rl_repo")

import numpy as np
import ml_dtypes

import concourse.bass as bass
import concourse.bacc as bacc
import concourse.tile as tile
import concourse.mybir as mybir
from concourse.bass_utils import run_bass_kernel_spmd

N_CORES = 8
B_FULL = 4096
B = B_FULL // N_CORES  # 512 batches per core
E = 64
H = 40
LP = 128               # compacted history slots per batch
NPAIR = B // 2         # 256 batch pairs per core
NGRP = 16              # pairs per group (one h2 psum tile)
NSLAB = 8              # kx/vw DMA slabs (32 pairs each)

BF16 = mybir.dt.bfloat16
FP8 = mybir.dt.float8e4
F32 = mybir.dt.float32
nbf16 = ml_dtypes.bfloat16
nfp8 = ml_dtypes.float8_e4m3fn


def build_nc():
    nc = bacc.Bacc()

    kx_d = nc.declare_dram_parameter("kx", [128, NPAIR * LP], FP8, isOutput=False)
    vw_d = nc.declare_dram_parameter("vw", [128, NPAIR * 80], FP8, isOutput=False)
    wf_d = nc.declare_dram_parameter("wf2", [80, 2], BF16, isOutput=False)
    vt_d = nc.declare_dram_parameter("vt", [B, E * LP], BF16, isOutput=False)
    madd_d = nc.declare_dram_parameter("madd", [B, LP], BF16, isOutput=False)
    out_d = nc.declare_dram_parameter("out", [B, E], F32, isOutput=True)

    Tanh = mybir.ActivationFunctionType.Tanh
    Exp = mybir.ActivationFunctionType.Exp
    Copy = mybir.ActivationFunctionType.Copy
    Alu = mybir.AluOpType
    X = mybir.AxisListType.X

    SLABC = 2 * NGRP * LP   # kx cols per slab (4096)
    SLABW = 2 * NGRP * 80   # vw cols per slab (2560)

    from contextlib import ExitStack

    with tile.TileContext(nc) as tc, ExitStack() as ctx:
        const = ctx.enter_context(tc.tile_pool(name="const", bufs=1))
        kxp = ctx.enter_context(tc.tile_pool(name="kxp", bufs=1))
        vwp = ctx.enter_context(tc.tile_pool(name="vwp", bufs=1))
        h2p = ctx.enter_context(tc.tile_pool(name="h2p", bufs=2, space="PSUM"))
        lgp = ctx.enter_context(tc.tile_pool(name="lgp", bufs=2, space="PSUM"))
        tp = ctx.enter_context(tc.tile_pool(name="tp", bufs=2))
        stp = ctx.enter_context(tc.tile_pool(name="stp", bufs=2))
        lmp = ctx.enter_context(tc.tile_pool(name="lmp", bufs=2))
        vtp = ctx.enter_context(tc.tile_pool(name="vtp", bufs=2))
        mp = ctx.enter_context(tc.tile_pool(name="mp", bufs=2))
        bp = ctx.enter_context(tc.tile_pool(name="bp", bufs=1))

        wf_t = const.tile([80, 2], BF16, tag="wf")
        nc.sync.dma_start(wf_t[:], wf_d[:])

        kx_t = {}
        vw_t = {}

        # variable slabs (in groups of 8 pairs): small first slabs so the
        # first matmul starts after ~256KB of DMA instead of 1.6MB
        SLAB_GROUPS = [2, 2, 4, 4, 4, 4, 4, 4, 4]
        SLAB_G0 = np.cumsum([0] + SLAB_GROUPS).tolist()

        def load_slab(s):
            ng = SLAB_GROUPS[s]
            g0 = SLAB_G0[s]
            kt = kxp.tile([128, ng * 8 * LP], FP8, tag=f"kx{s}", name=f"kx{s}")
            nc.sync.dma_start(kt[:], kx_d[:, g0 * 8 * LP:(g0 + ng) * 8 * LP])
            kx_t[s] = kt
            wt = vwp.tile([128, ng * 8 * 80], FP8, tag=f"vw{s}", name=f"vw{s}")
            nc.sync.dma_start(wt[:], vw_d[:, g0 * 8 * 80:(g0 + ng) * 8 * 80])
            vw_t[s] = wt

        qdat = {}

        def load_quarter(qq, chunk):
            # same (sync) ring as the kx/vw slabs so arrival order matches
            # consumption order, but in 512KB chunks interleaved between
            # slab loads so the slab stream never stalls behind a 2MB blob
            if chunk == 0:
                vt_t = vtp.tile([128, E * LP], BF16, tag="vt", name=f"vt{qq}")
                md_t = mp.tile([128, LP], BF16, tag="md", name=f"md{qq}")
                nc.gpsimd.dma_start(md_t[:], madd_d[qq * 128:(qq + 1) * 128, :])
                qdat[qq] = (vt_t, md_t)
            vt_t = qdat[qq][0]
            c0 = chunk * (E * LP // 4)
            c1 = (chunk + 1) * (E * LP // 4)
            nc.gpsimd.dma_start(vt_t[:, c0:c1], vt_d[qq * 128:(qq + 1) * 128, c0:c1])

        pend = {}

        def phase_b_pre(qq, lm_t):
            vt_t, md_t = qdat.pop(qq)
            ladj = bp.tile([128, LP], F32, tag="ladj", name=f"ladj{qq}")
            nc.vector.tensor_tensor(ladj[:], lm_t[:], md_t[:], Alu.add)
            pend[qq] = (vt_t, ladj)

        def phase_b(qq):
            vt_t, ladj = pend.pop(qq)
            p_t = bp.tile([128, LP], BF16, tag="p", name=f"p{qq}")
            z_t = bp.tile([128, 1], F32, tag="z", name=f"z{qq}")
            nc.scalar.activation(p_t[:], ladj[:], Exp, accum_out=z_t[:])

            w1 = bp.tile([128, E * LP], BF16, tag="w1", name=f"w1{qq}")
            p_b = p_t[:].rearrange("p (o l) -> p o l", o=1).broadcast_to([128, E, LP])
            nc.vector.tensor_tensor(
                w1[:].rearrange("p (e l) -> p e l", e=E),
                vt_t[:].rearrange("p (e l) -> p e l", e=E),
                p_b, Alu.mult,
            )
            w2 = bp.tile([128, E * LP // 2], BF16, tag="w2", name=f"w2{qq}")
            w1v = w1[:].rearrange("p (e l) -> p e l", e=E)
            nc.vector.tensor_tensor(
                w2[:].rearrange("p (e l) -> p e l", e=E),
                w1v[:, :, 0:LP // 2], w1v[:, :, LP // 2:LP], Alu.add,
            )
            w3 = bp.tile([128, E * LP // 4], BF16, tag="w3", name=f"w3{qq}")
            w2v = w2[:].rearrange("p (e l) -> p e l", e=E)
            nc.vector.tensor_tensor(
                w3[:].rearrange("p (e l) -> p e l", e=E),
                w2v[:, :, 0:LP // 4], w2v[:, :, LP // 4:LP // 2], Alu.add,
            )
            acc = bp.tile([128, E], F32, tag="acc", name=f"acc{qq}")
            nc.vector.tensor_reduce(
                acc[:], w3[:].rearrange("p (e l) -> p e l", e=E), axis=X, op=Alu.add)
            rz = bp.tile([128, 1], F32, tag="rz", name=f"rz{qq}")
            nc.vector.reciprocal(rz[:], z_t[:])
            o_t = bp.tile([128, E], F32, tag="o", name=f"o{qq}")
            nc.vector.tensor_scalar_mul(o_t[:], acc[:], rz[:])
            nc.gpsimd.dma_start(out_d[qq * 128:(qq + 1) * 128, :], o_t[:])

        load_slab(0)
        st_t = None
        lg_t = None
        GP = 8  # pairs per h2 group ([80, 1024] f32 = 2 psum banks)
        slab_of_group = []
        for si, ng in enumerate(SLAB_GROUPS):
            slab_of_group += [si] * ng
        for g in range(32):
            if g % 8 == 2 and g // 8 - 1 in pend:
                phase_b(g // 8 - 1)
            s = slab_of_group[g]
            if g == SLAB_G0[s] and s + 1 < len(SLAB_GROUPS):
                load_slab(s + 1)
            if 4 <= g < 8:
                load_quarter(0, g - 4)
            elif g >= 8 and 2 <= g % 8 < 6:
                load_quarter(g // 8, g % 8 - 2)

            kxs, vws = kx_t[s], vw_t[s]
            h2_t = h2p.tile([80, GP * LP], F32, tag="h2", name=f"h2_{g}")
            for pp in range(GP):
                rr = (g - SLAB_G0[s]) * GP + pp  # pair within slab
                nc.tensor.matmul(
                    h2_t[0:80, pp * LP:(pp + 1) * LP],
                    vws[:, rr * 80:rr * 80 + 80],
                    kxs[:, rr * LP:(rr + 1) * LP],
                    start=True, stop=True,
                )
            t_t = tp.tile([80, GP * LP], BF16, tag="t", name=f"t_{g}")
            nc.scalar.activation(t_t[:], h2_t[:], Tanh, scale=0.5)

            if g % 2 == 0:
                lg_t = lgp.tile([98, 512], F32, tag="lg", name=f"lg_{g // 2}")
            for j in range(2):
                ss = 2 * (g % 2) + j
                nc.tensor.matmul(
                    lg_t[32 * ss:32 * ss + 2, 0:512],
                    wf_t[:], t_t[:, 512 * j:512 * (j + 1)],
                    start=True, stop=True, tile_position=(0, 32 * ss),
                )
            if g % 2 == 1:
                qq, gq = g // 8, (g // 2) % 4
                if gq == 0:
                    st_t = stp.tile([98, 4 * 512], BF16, tag="st", name=f"st{qq}")
                nc.scalar.activation(
                    st_t[:, 512 * gq:512 * (gq + 1)], lg_t[:], Copy)

                if gq == 3:
                    lm_t = lmp.tile([128, LP], BF16, tag="lm", name=f"lm{qq}")
                    # rows {32s, 32s+1} unfold to batch-major rows
                    # 32s..32s+32 in one DMA (row-major both sides);
                    # last quarter rides the idle scalar HWDGE ring to cut
                    # the end-of-kernel staging latency
                    deng = nc.scalar if qq == 3 else nc.gpsimd
                    for ss in range(4):
                        deng.dma_start(
                            lm_t[32 * ss:32 * ss + 32, :],
                            st_t[32 * ss:32 * ss + 2, :])
                    phase_b_pre(qq, lm_t)
        phase_b(3)

    if not nc.is_finalized():
        nc.finalize()
    return nc


def host_prep(q, k, v, mask, W1, b1, W2, b2, Wf, bf):
    """Fold weights per batch, compact masked tokens, build device layouts."""
    q2 = q[:, 0, :].astype(np.float32)
    W1 = W1.astype(np.float32); W2 = W2.astype(np.float32)
    P = (W1[64:128] - W1[128:192]) @ W2                     # [64,40]
    Q = W1[192:256] @ W2                                    # [64,40]
    A2 = (W1[0:64] + W1[128:192]) @ W2
    c0 = b1.astype(np.float32) @ W2 + b2.astype(np.float32)
    r = q2 @ A2 + c0                                        # [Bf,40]
    V = P[None] + q2[:, :, None] * Q[None]                  # [Bf,64,40]
    G = np.einsum('beh,bei->bhi', V, V)
    y = np.linalg.solve(G, r[:, :, None])
    s = np.einsum('beh,bhx->be', V, y)                      # [Bf,64]

    m = mask[:, :, 0]
    order = np.argsort(m, axis=1, kind='stable')[:, :LP]
    nvalid = (~m).sum(1)
    assert nvalid.max() <= LP, f"batch with {nvalid.max()} unmasked tokens"
    validc = np.arange(LP)[None, :] < nvalid[:, None]       # [Bf,LP]
    kc = np.take_along_axis(k.astype(np.float32), order[:, :, None], 1)
    vc = np.take_along_axis(v.astype(np.float32), order[:, :, None], 1)
    kc = np.where(validc[..., None], kc + s[:, None, :], 0.0)
    vc = np.where(validc[..., None], vc, 0.0)
    maddf = np.where(validc, np.float32(0.0), np.float32(-30.0)).astype(nbf16)

    # core-local batch <-> (pair r, stream sig) map
    b = np.arange(B)
    qq = b // 128; t = b % 128
    s2s = t // 16; s_ = s2s // 2; sig = s2s % 2
    g_ = (t % 16) // 4; cb = t % 4
    r_ = 64 * qq + 16 * g_ + 4 * s_ + cb
    A_idx = np.empty(NPAIR, np.int64); B_idx = np.empty(NPAIR, np.int64)
    A_idx[r_[sig == 0]] = b[sig == 0]
    B_idx[r_[sig == 1]] = b[sig == 1]

    in_maps = []
    for c in range(N_CORES):
        sl = slice(c * B, (c + 1) * B)
        kcc, Vc = kc[sl], V[sl]
        kx = np.empty((128, NPAIR * LP), np.float32)
        kx[0:64] = kcc[A_idx].transpose(2, 0, 1).reshape(64, -1)
        kx[64:128] = kcc[B_idx].transpose(2, 0, 1).reshape(64, -1)
        vw3 = np.zeros((NPAIR, 128, 80), np.float32)
        vw3[:, 0:64, 0:40] = Vc[A_idx]
        vw3[:, 64:128, 40:80] = Vc[B_idx]
        vw = vw3.transpose(1, 0, 2).reshape(128, NPAIR * 80)
        vt = np.ascontiguousarray(vc[sl].transpose(0, 2, 1)).reshape(B, E * LP)
        wf2 = np.zeros((80, 2), np.float32)
        wf2[0:40, 0] = 0.5 * Wf[:, 0]
        wf2[40:80, 1] = 0.5 * Wf[:, 0]
        in_maps.append({
            "kx": np.ascontiguousarray(kx).astype(nfp8),
            "vw": np.ascontiguousarray(vw).astype(nfp8),
            "wf2": wf2.astype(nbf16),
            "vt": vt.astype(nbf16),
            "madd": np.ascontiguousarray(maddf[sl]),
        })
    return in_maps


_CACHE = {}


def run_on_device(in_maps, trace=False):
    if "nc" not in _CACHE:
        _CACHE["nc"] = build_nc()
    nc = _CACHE["nc"]
    res = run_bass_kernel_spmd(nc, in_maps, core_ids=list(range(N_CORES)),
                               trace=trace)
    return res


def kernel(q, k, v, mask, W1, b1, W2, b2, Wf, bf):
    in_maps = host_prep(q, k, v, mask, W1, b1, W2, b2, Wf, bf)
    res = run_on_device(in_maps)
    out = np.concatenate([res.results[c]["out"] for c in range(N_CORES)], axis=0)
    return out.astype(np.float32)


# revision 24
# speedup vs baseline: 1.2584x; 1.0920x over previous
"""Trainium2 Bass kernel for nn_AttentionLayer (sparse_attention).

Math (per batch b, history l):
    info = [q, k, q-k, q*k] @ W1 + b1 ; @ W2 + b2 ; sigmoid ; @ Wf + bf
    score = softmax(where(mask, -inf, logit), axis=l)
    out   = sum_l score * v

Host-side algebra (exact up to fp assoc):
  - No nonlinearity between W1/W2  =>  h2 = k@P + (q*k)@Q + r_b
        P = (W1b-W1c)@W2, Q = W1d@W2, r_b = q_b@(W1a+W1c)@W2 + b1@W2 + b2
  - Fold q into per-batch weights: h2 = k @ V_b + r_b,  V_b = P + diag(q_b) Q
  - Fold r_b into k: solve s_b @ V_b = r_b (least-norm), ship k + s_b
  - sigmoid(x)@Wf = tanh(x/2)@(Wf/2) + const; const cancels in softmax
  - MASK COMPACTION: masked tokens (score exactly 0) are dropped on host;
    batches are sorted by unmasked count and binned into 4 quarters with
    slot counts LPS=[128,112,104,96] (pads: k=0 -> logit 0, madd=-30, v=0).
  - k-stream and folded weights ship in fp8e4m3 (softmax smooths the
    quantization); v ships bf16 (it hits the output linearly).
Device layout: token-major 2-stream columns (batch-pair -> LP cols,
partitions 0:64 = stream-A E-dims, 64:128 = stream-B). One block-diagonal
[128,80] fp8 matmul per pair -> h2 [80,LP] (A h2 parts 0:40, B 40:80); tanh
(scale .5) -> t bf16; wf matmuls [80,2] write logits into 4 PSUM partition
strips (32s, 32s+1); ACT-copy evacuates [98, 4LP] to bf16 staging; 4 DMAs
per quarter unfold logits batch-major [128, LP]; softmax + p@v on DVE
(exp w/ accum z on ACT; mult + 2 folds + reduce + scale).
"""

import sys

sys.path.insert(0, "/opt/trn_rl_repo")

import numpy as np
import ml_dtypes

import concourse.bass as bass
import concourse.bacc as bacc
import concourse.tile as tile
import concourse.mybir as mybir
from concourse.bass_utils import run_bass_kernel_spmd

N_CORES = 8
B_FULL = 4096
B = B_FULL // N_CORES   # 512 batches per core
E = 64
H = 40
LPS = [128, 112, 104, 96]   # history slots per quarter (sorted batches)
KB = [0]                    # kx col base per quarter
for _lp in LPS:
    KB.append(KB[-1] + 64 * _lp)
NKX = KB[4]                 # total kx columns (28160)
NPAIR = 256
GP = 8                      # pairs per group
# slabs: (quarter, first group in quarter, n groups)
SLABS = [(0, 0, 2), (0, 2, 2), (0, 4, 4),
         (1, 0, 4), (1, 4, 4), (2, 0, 4), (2, 4, 4), (3, 0, 4), (3, 4, 4)]

BF16 = mybir.dt.bfloat16
FP8 = mybir.dt.float8e4
F32 = mybir.dt.float32
nbf16 = ml_dtypes.bfloat16
nfp8 = ml_dtypes.float8_e4m3fn


def build_nc():
    nc = bacc.Bacc()

    kx_d = nc.declare_dram_parameter("kx", [128, NKX], FP8, isOutput=False)
    vw_d = nc.declare_dram_parameter("vw", [128, NPAIR * 80], FP8, isOutput=False)
    wf_d = nc.declare_dram_parameter("wf2", [80, 2], BF16, isOutput=False)
    vt_d = [nc.declare_dram_parameter(f"vt{q}", [128, E * LPS[q]], BF16,
                                      isOutput=False) for q in range(4)]
    madd_d = [nc.declare_dram_parameter(f"madd{q}", [128, LPS[q]], BF16,
                                        isOutput=False) for q in range(4)]
    out_d = nc.declare_dram_parameter("out", [B, E], F32, isOutput=True)

    Tanh = mybir.ActivationFunctionType.Tanh
    Exp = mybir.ActivationFunctionType.Exp
    Copy = mybir.ActivationFunctionType.Copy
    Alu = mybir.AluOpType
    X = mybir.AxisListType.X

    from contextlib import ExitStack

    with tile.TileContext(nc) as tc, ExitStack() as ctx:
        const = ctx.enter_context(tc.tile_pool(name="const", bufs=1))
        kxp = ctx.enter_context(tc.tile_pool(name="kxp", bufs=1))
        vwp = ctx.enter_context(tc.tile_pool(name="vwp", bufs=1))
        h2p = ctx.enter_context(tc.tile_pool(name="h2p", bufs=2, space="PSUM"))
        lgp = ctx.enter_context(tc.tile_pool(name="lgp", bufs=2, space="PSUM"))
        tp = ctx.enter_context(tc.tile_pool(name="tp", bufs=2))
        stp = ctx.enter_context(tc.tile_pool(name="stp", bufs=2))
        lmp = ctx.enter_context(tc.tile_pool(name="lmp", bufs=2))
        vtp = ctx.enter_context(tc.tile_pool(name="vtp", bufs=2))
        mp = ctx.enter_context(tc.tile_pool(name="mp", bufs=2))
        bp = ctx.enter_context(tc.tile_pool(name="bp", bufs=1))

        wf_t = const.tile([80, 2], BF16, tag="wf")
        nc.sync.dma_start(wf_t[:], wf_d[:])

        kx_t = {}
        vw_t = {}

        # global group -> slab index; global first group of each slab
        g2slab = {}
        slab_g0 = []
        for si, (sq, g0q, ng) in enumerate(SLABS):
            slab_g0.append(8 * sq + g0q)
            for j in range(ng):
                g2slab[8 * sq + g0q + j] = si

        def load_slab(si):
            sq, g0q, ng = SLABS[si]
            lp = LPS[sq]
            c0 = KB[sq] + g0q * GP * lp
            ncol = ng * GP * lp
            kt = kxp.tile([128, ncol], FP8, tag=f"kx{si}", name=f"kx{si}")
            nc.sync.dma_start(kt[:], kx_d[:, c0:c0 + ncol])
            kx_t[si] = kt
            r0 = 64 * sq + g0q * GP
            wt = vwp.tile([128, ng * GP * 80], FP8, tag=f"vw{si}", name=f"vw{si}")
            nc.sync.dma_start(wt[:], vw_d[:, r0 * 80:(r0 + ng * GP) * 80])
            vw_t[si] = wt

        qdat = {}
        pend = {}

        def load_quarter(qq, chunk):
            lp = LPS[qq]
            if chunk == 0:
                vt_t = vtp.tile([128, E * 128], BF16, tag="vt", name=f"vt{qq}")
                md_t = mp.tile([128, 128], BF16, tag="md", name=f"md{qq}")
                nc.gpsimd.dma_start(md_t[:, 0:lp], madd_d[qq][:])
                qdat[qq] = (vt_t, md_t)
            vt_t = qdat[qq][0]
            c0 = chunk * (E * lp // 4)
            c1 = (chunk + 1) * (E * lp // 4)
            nc.gpsimd.dma_start(vt_t[:, c0:c1], vt_d[qq][:, c0:c1])

        def phase_b_pre(qq, lm_t):
            vt_t, md_t = qdat.pop(qq)
            lp = LPS[qq]
            ladj = bp.tile([128, 128], F32, tag="ladj", name=f"ladj{qq}")
            nc.vector.tensor_tensor(
                ladj[:, 0:lp], lm_t[:, 0:lp], md_t[:, 0:lp], Alu.add)
            pend[qq] = (vt_t, ladj)

        def phase_b(qq):
            vt_t, ladj = pend.pop(qq)
            lp = LPS[qq]
            p_t = bp.tile([128, 128], BF16, tag="p", name=f"p{qq}")
            z_t = bp.tile([128, 1], F32, tag="z", name=f"z{qq}")
            nc.scalar.activation(p_t[:, 0:lp], ladj[:, 0:lp], Exp,
                                 accum_out=z_t[:])

            w1 = bp.tile([128, E * 128], BF16, tag="w1", name=f"w1{qq}")
            p_b = p_t[:, 0:lp].rearrange("p (o l) -> p o l", o=1) \
                              .broadcast_to([128, E, lp])
            nc.vector.tensor_tensor(
                w1[:, 0:E * lp].rearrange("p (e l) -> p e l", e=E),
                vt_t[:, 0:E * lp].rearrange("p (e l) -> p e l", e=E),
                p_b, Alu.mult,
            )
            w2 = bp.tile([128, E * 64], BF16, tag="w2", name=f"w2{qq}")
            w1v = w1[:, 0:E * lp].rearrange("p (e l) -> p e l", e=E)
            nc.vector.tensor_tensor(
                w2[:, 0:E * lp // 2].rearrange("p (e l) -> p e l", e=E),
                w1v[:, :, 0:lp // 2], w1v[:, :, lp // 2:lp], Alu.add,
            )
            w3 = bp.tile([128, E * 32], BF16, tag="w3", name=f"w3{qq}")
            w2v = w2[:, 0:E * lp // 2].rearrange("p (e l) -> p e l", e=E)
            nc.vector.tensor_tensor(
                w3[:, 0:E * lp // 4].rearrange("p (e l) -> p e l", e=E),
                w2v[:, :, 0:lp // 4], w2v[:, :, lp // 4:lp // 2], Alu.add,
            )
            acc = bp.tile([128, E], F32, tag="acc", name=f"acc{qq}")
            nc.vector.tensor_reduce(
                acc[:], w3[:, 0:E * lp // 4].rearrange("p (e l) -> p e l", e=E),
                axis=X, op=Alu.add)
            rz = bp.tile([128, 1], F32, tag="rz", name=f"rz{qq}")
            nc.vector.reciprocal(rz[:], z_t[:])
            o_t = bp.tile([128, E], F32, tag="o", name=f"o{qq}")
            nc.vector.tensor_scalar_mul(o_t[:], acc[:], rz[:])
            nc.gpsimd.dma_start(out_d[qq * 128:(qq + 1) * 128, :], o_t[:])

        load_slab(0)
        st_t = None
        lg_t = None
        for g in range(32):
            if g % 8 == 2 and g // 8 - 1 in pend:
                phase_b(g // 8 - 1)
            si = g2slab[g]
            if g == slab_g0[si] and si + 1 < len(SLABS):
                load_slab(si + 1)
            if 4 <= g < 8:
                load_quarter(0, g - 4)
            elif g >= 8 and 2 <= g % 8 < 6:
                load_quarter(g // 8, g % 8 - 2)

            qq = g // 8
            lp = LPS[qq]
            gc = GP * lp                      # group columns
            kxs, vws = kx_t[si], vw_t[si]
            sq, g0q, _ = SLABS[si]
            h2_t = h2p.tile([80, GP * 128], F32, tag="h2", name=f"h2_{g}")
            for pp in range(GP):
                rr = (g - 8 * sq - g0q) * GP + pp   # pair within slab
                nc.tensor.matmul(
                    h2_t[0:80, pp * lp:(pp + 1) * lp],
                    vws[:, rr * 80:rr * 80 + 80],
                    kxs[:, rr * lp:(rr + 1) * lp],
                    start=True, stop=True,
                )
            t_t = tp.tile([80, GP * 128], BF16, tag="t", name=f"t_{g}")
            nc.scalar.activation(
                t_t[:, 0:gc], h2_t[0:80, 0:gc], Tanh, scale=0.5)

            if g % 2 == 0:
                lg_t = lgp.tile([98, 512], F32, tag="lg", name=f"lg_{g // 2}")
            for j in range(2):
                ss = 2 * (g % 2) + j
                nc.tensor.matmul(
                    lg_t[32 * ss:32 * ss + 2, 0:4 * lp],
                    wf_t[:], t_t[:, j * 4 * lp:(j + 1) * 4 * lp],
                    start=True, stop=True, tile_position=(0, 32 * ss),
                )
            if g % 2 == 1:
                gq = (g // 2) % 4
                if gq == 0:
                    st_t = stp.tile([98, 4 * 512], BF16, tag="st", name=f"st{qq}")
                nc.scalar.activation(
                    st_t[:, 4 * lp * gq:4 * lp * (gq + 1)],
                    lg_t[:, 0:4 * lp], Copy)

                if gq == 3:
                    lm_t = lmp.tile([128, 128], BF16, tag="lm", name=f"lm{qq}")
                    deng = nc.scalar if qq == 3 else nc.gpsimd
                    for ss in range(4):
                        deng.dma_start(
                            lm_t[32 * ss:32 * ss + 32, 0:lp],
                            st_t[32 * ss:32 * ss + 2, 0:16 * lp])
                    phase_b_pre(qq, lm_t)
        phase_b(3)

    if not nc.is_finalized():
        nc.finalize()
    return nc


def host_prep(q, k, v, mask, W1, b1, W2, b2, Wf, bf):
    """Fold weights per batch, compact masked tokens, build device layouts."""
    q2 = q[:, 0, :].astype(np.float32)
    W1 = W1.astype(np.float32); W2 = W2.astype(np.float32)
    P = (W1[64:128] - W1[128:192]) @ W2
    Q = W1[192:256] @ W2
    A2 = (W1[0:64] + W1[128:192]) @ W2
    c0 = b1.astype(np.float32) @ W2 + b2.astype(np.float32)
    r = q2 @ A2 + c0
    V = P[None] + q2[:, :, None] * Q[None]                  # [Bf,64,40]
    G = np.einsum('beh,bei->bhi', V, V)
    y = np.linalg.solve(G, r[:, :, None])
    s = np.einsum('beh,bhx->be', V, y)

    m = mask[:, :, 0]
    order = np.argsort(m, axis=1, kind='stable')[:, :128]
    nvalid = (~m).sum(1)
    validc = np.arange(128)[None, :] < nvalid[:, None]
    kc = np.take_along_axis(k.astype(np.float32), order[:, :, None], 1)
    vc = np.take_along_axis(v.astype(np.float32), order[:, :, None], 1)
    kc = np.where(validc[..., None], kc + s[:, None, :], 0.0)
    vc = np.where(validc[..., None], vc, 0.0)
    maddf = np.where(validc, np.float32(0.0), np.float32(-30.0)).astype(nbf16)

    # pair rq -> within-quarter positions of its A/B batches
    rqi = np.arange(64)
    tA = 32 * ((rqi % 16) // 4) + 4 * (rqi // 16) + rqi % 4
    tB = tA + 16

    in_maps = []
    invs = []
    for c in range(N_CORES):
        sl = slice(c * B, (c + 1) * B)
        ncore = nvalid[sl]
        ordr = np.argsort(-ncore, kind='stable')   # desc by unmasked count
        invs.append(ordr)
        kcc, vcc, Vc = kc[sl], vc[sl], V[sl]
        kx = np.zeros((128, NKX), np.float32)
        vw3 = np.zeros((NPAIR, 128, 80), np.float32)
        imap = {}
        for qq in range(4):
            lp = LPS[qq]
            ranks = ordr[128 * qq:128 * (qq + 1)]
            assert ncore[ranks].max() <= lp, \
                f"core {c} q{qq}: {ncore[ranks].max()} > {lp}"
            Ab = ranks[tA]
            Bb = ranks[tB]
            kx[0:64, KB[qq]:KB[qq + 1]] = \
                kcc[Ab][:, :lp].transpose(2, 0, 1).reshape(64, -1)
            kx[64:128, KB[qq]:KB[qq + 1]] = \
                kcc[Bb][:, :lp].transpose(2, 0, 1).reshape(64, -1)
            vw3[64 * qq:64 * (qq + 1), 0:64, 0:40] = Vc[Ab]
            vw3[64 * qq:64 * (qq + 1), 64:128, 40:80] = Vc[Bb]
            imap[f"vt{qq}"] = np.ascontiguousarray(
                vcc[ranks][:, :lp].transpose(0, 2, 1)
            ).reshape(128, E * lp).astype(nbf16)
            imap[f"madd{qq}"] = np.ascontiguousarray(maddf[sl][ranks][:, :lp])
        vw = vw3.transpose(1, 0, 2).reshape(128, NPAIR * 80)
        wf2 = np.zeros((80, 2), np.float32)
        wf2[0:40, 0] = 0.5 * Wf[:, 0]
        wf2[40:80, 1] = 0.5 * Wf[:, 0]
        imap["kx"] = np.ascontiguousarray(kx).astype(nfp8)
        imap["vw"] = np.ascontiguousarray(vw).astype(nfp8)
        imap["wf2"] = wf2.astype(nbf16)
        in_maps.append(imap)
    return in_maps, invs


_CACHE = {}


def run_on_device(in_maps, trace=False):
    if "nc" not in _CACHE:
        _CACHE["nc"] = build_nc()
    nc = _CACHE["nc"]
    res = run_bass_kernel_spmd(nc, in_maps, core_ids=list(range(N_CORES)),
                               trace=trace)
    return res


def gather_out(res, invs):
    outs = []
    for c in range(N_CORES):
        oc = np.empty((B, E), np.float32)
        oc[invs[c]] = res.results[c]["out"]
        outs.append(oc)
    return np.concatenate(outs, axis=0)


def kernel(q, k, v, mask, W1, b1, W2, b2, Wf, bf):
    in_maps, invs = host_prep(q, k, v, mask, W1, b1, W2, b2, Wf, bf)
    res = run_on_device(in_maps)
    return gather_out(res, invs).astype(np.float32)


# revision 26
# speedup vs baseline: 1.2824x; 1.0191x over previous
"""Trainium2 Bass kernel for nn_AttentionLayer (sparse_attention).

Math (per batch b, history l):
    info = [q, k, q-k, q*k] @ W1 + b1 ; @ W2 + b2 ; sigmoid ; @ Wf + bf
    score = softmax(where(mask, -inf, logit), axis=l)
    out   = sum_l score * v

Host-side algebra (exact up to fp assoc):
  - No nonlinearity between W1/W2  =>  h2 = k@P + (q*k)@Q + r_b
        P = (W1b-W1c)@W2, Q = W1d@W2, r_b = q_b@(W1a+W1c)@W2 + b1@W2 + b2
  - Fold q into per-batch weights: h2 = k @ V_b + r_b,  V_b = P + diag(q_b) Q
  - Fold r_b into k: solve s_b @ V_b = r_b (least-norm), ship k + s_b
  - sigmoid(x)@Wf = tanh(x/2)@(Wf/2) + const; const cancels in softmax
  - MASK COMPACTION: masked tokens (score exactly 0) are dropped on host;
    batches are sorted by unmasked count and binned into 4 quarters with
    slot counts LPS=[128,112,104,96] (pads: k=0 -> logit 0, madd=-30, v=0).
  - k-stream and folded weights ship in fp8e4m3 (softmax smooths the
    quantization); v ships bf16 (it hits the output linearly).
Device layout: token-major 2-stream columns (batch-pair -> LP cols,
partitions 0:64 = stream-A E-dims, 64:128 = stream-B). One block-diagonal
[128,80] fp8 matmul per pair -> h2 [80,LP] (A h2 parts 0:40, B 40:80); tanh
(scale .5) -> t bf16; wf matmuls [80,2] write logits into 4 PSUM partition
strips (32s, 32s+1); ACT-copy evacuates [98, 4LP] to bf16 staging; 4 DMAs
per quarter unfold logits batch-major [128, LP]; softmax + p@v on DVE
(exp w/ accum z on ACT; mult + 2 folds + reduce + scale).
"""

import sys

sys.path.insert(0, "/opt/trn_rl_repo")

import numpy as np
import ml_dtypes

import concourse.bass as bass
import concourse.bacc as bacc
import concourse.tile as tile
import concourse.mybir as mybir
from concourse.bass_utils import run_bass_kernel_spmd

N_CORES = 8
B_FULL = 4096
B = B_FULL // N_CORES   # 512 batches per core
E = 64
E2 = 65             # + ones-row: z rides the same reduction
H = 40
LPS = [128, 112, 104, 96]   # history slots per quarter (sorted batches)
KB = [0]                    # kx col base per quarter
for _lp in LPS:
    KB.append(KB[-1] + 64 * _lp)
NKX = KB[4]                 # total kx columns (28160)
NPAIR = 256
GP = 8                      # pairs per group
# slabs: (quarter, first group in quarter, n groups)
SLABS = [(0, 0, 1), (0, 1, 1), (0, 2, 1), (0, 3, 1), (0, 4, 4),
         (1, 0, 4), (1, 4, 4), (2, 0, 4), (2, 4, 4), (3, 0, 4), (3, 4, 4)]

BF16 = mybir.dt.bfloat16
FP8 = mybir.dt.float8e4
F32 = mybir.dt.float32
nbf16 = ml_dtypes.bfloat16
nfp8 = ml_dtypes.float8_e4m3fn


def build_nc():
    nc = bacc.Bacc()

    kx_d = nc.declare_dram_parameter("kx", [128, NKX], FP8, isOutput=False)
    vw_d = nc.declare_dram_parameter("vw", [128, NPAIR * 80], FP8, isOutput=False)
    wf_d = nc.declare_dram_parameter("wf2", [80, 2], BF16, isOutput=False)
    vt_d = [nc.declare_dram_parameter(f"vt{q}", [128, E2 * LPS[q]], BF16,
                                      isOutput=False) for q in range(4)]
    out_d = nc.declare_dram_parameter("out", [B, E], F32, isOutput=True)

    Tanh = mybir.ActivationFunctionType.Tanh
    Exp = mybir.ActivationFunctionType.Exp
    Copy = mybir.ActivationFunctionType.Copy
    Alu = mybir.AluOpType
    X = mybir.AxisListType.X

    from contextlib import ExitStack

    with tile.TileContext(nc) as tc, ExitStack() as ctx:
        const = ctx.enter_context(tc.tile_pool(name="const", bufs=1))
        kxp = ctx.enter_context(tc.tile_pool(name="kxp", bufs=1))
        vwp = ctx.enter_context(tc.tile_pool(name="vwp", bufs=1))
        h2p = ctx.enter_context(tc.tile_pool(name="h2p", bufs=2, space="PSUM"))
        lgp = ctx.enter_context(tc.tile_pool(name="lgp", bufs=2, space="PSUM"))
        tp = ctx.enter_context(tc.tile_pool(name="tp", bufs=2))
        stp = ctx.enter_context(tc.tile_pool(name="stp", bufs=2))
        lmp = ctx.enter_context(tc.tile_pool(name="lmp", bufs=2))
        vtp = ctx.enter_context(tc.tile_pool(name="vtp", bufs=2))
        bp = ctx.enter_context(tc.tile_pool(name="bp", bufs=1))

        wf_t = const.tile([80, 2], BF16, tag="wf")
        nc.sync.dma_start(wf_t[:], wf_d[:])

        kx_t = {}
        vw_t = {}

        # global group -> slab index; global first group of each slab
        g2slab = {}
        slab_g0 = []
        for si, (sq, g0q, ng) in enumerate(SLABS):
            slab_g0.append(8 * sq + g0q)
            for j in range(ng):
                g2slab[8 * sq + g0q + j] = si

        def load_slab(si):
            sq, g0q, ng = SLABS[si]
            lp = LPS[sq]
            c0 = KB[sq] + g0q * GP * lp
            ncol = ng * GP * lp
            kt = kxp.tile([128, ncol], FP8, tag=f"kx{si}", name=f"kx{si}")
            nc.sync.dma_start(kt[:], kx_d[:, c0:c0 + ncol])
            kx_t[si] = kt
            r0 = 64 * sq + g0q * GP
            wt = vwp.tile([128, ng * GP * 80], FP8, tag=f"vw{si}", name=f"vw{si}")
            nc.sync.dma_start(wt[:], vw_d[:, r0 * 80:(r0 + ng * GP) * 80])
            vw_t[si] = wt

        qdat = {}
        pend = {}

        def load_quarter(qq, chunk):
            lp = LPS[qq]
            if chunk == 0:
                vt_t = vtp.tile([128, E2 * 128], BF16, tag="vt", name=f"vt{qq}")
                qdat[qq] = vt_t
            vt_t = qdat[qq]
            nck = E2 * lp // 4 if chunk < 3 else E2 * lp - 3 * (E2 * lp // 4)
            c0 = chunk * (E2 * lp // 4)
            nc.gpsimd.dma_start(vt_t[:, c0:c0 + nck], vt_d[qq][:, c0:c0 + nck])

        def phase_b_pre(qq, lm_t):
            pend[qq] = (qdat.pop(qq), lm_t)

        def phase_b(qq):
            vt_t, lm_t = pend.pop(qq)
            lp = LPS[qq]
            p_t = bp.tile([128, 128], BF16, tag="p", name=f"p{qq}")
            nc.scalar.activation(p_t[:, 0:lp], lm_t[:, 0:lp], Exp)

            w1 = bp.tile([128, E2 * 128], BF16, tag="w1", name=f"w1{qq}")
            p_b = p_t[:, 0:lp].rearrange("p (o l) -> p o l", o=1) \
                              .broadcast_to([128, E2, lp])
            nc.vector.tensor_tensor(
                w1[:, 0:E2 * lp].rearrange("p (e l) -> p e l", e=E2),
                vt_t[:, 0:E2 * lp].rearrange("p (e l) -> p e l", e=E2),
                p_b, Alu.mult,
            )
            w2 = bp.tile([128, E2 * 64], BF16, tag="w2", name=f"w2{qq}")
            w1v = w1[:, 0:E2 * lp].rearrange("p (e l) -> p e l", e=E2)
            nc.vector.tensor_tensor(
                w2[:, 0:E2 * lp // 2].rearrange("p (e l) -> p e l", e=E2),
                w1v[:, :, 0:lp // 2], w1v[:, :, lp // 2:lp], Alu.add,
            )
            w3 = bp.tile([128, E2 * 32], BF16, tag="w3", name=f"w3{qq}")
            w2v = w2[:, 0:E2 * lp // 2].rearrange("p (e l) -> p e l", e=E2)
            nc.vector.tensor_tensor(
                w3[:, 0:E2 * lp // 4].rearrange("p (e l) -> p e l", e=E2),
                w2v[:, :, 0:lp // 4], w2v[:, :, lp // 4:lp // 2], Alu.add,
            )
            acc = bp.tile([128, E2], F32, tag="acc", name=f"acc{qq}")
            nc.vector.tensor_reduce(
                acc[:], w3[:, 0:E2 * lp // 4].rearrange("p (e l) -> p e l", e=E2),
                axis=X, op=Alu.add)
            rz = bp.tile([128, 1], F32, tag="rz", name=f"rz{qq}")
            nc.vector.reciprocal(rz[:], acc[:, E:E2])
            o_t = bp.tile([128, E], F32, tag="o", name=f"o{qq}")
            nc.vector.tensor_scalar_mul(o_t[:], acc[:, 0:E], rz[:])
            nc.gpsimd.dma_start(out_d[qq * 128:(qq + 1) * 128, :], o_t[:])

        load_slab(0)
        st_t = None
        lg_t = None
        for g in range(32):
            if g % 8 == 2 and g // 8 - 1 in pend:
                phase_b(g // 8 - 1)
            si = g2slab[g]
            if g == slab_g0[si] and si + 1 < len(SLABS):
                load_slab(si + 1)
            if 4 <= g < 8:
                load_quarter(0, g - 4)
            elif g >= 8 and 2 <= g % 8 < 6:
                load_quarter(g // 8, g % 8 - 2)

            qq = g // 8
            lp = LPS[qq]
            gc = GP * lp                      # group columns
            kxs, vws = kx_t[si], vw_t[si]
            sq, g0q, _ = SLABS[si]
            h2_t = h2p.tile([80, GP * 128], F32, tag="h2", name=f"h2_{g}")
            for pp in range(GP):
                rr = (g - 8 * sq - g0q) * GP + pp   # pair within slab
                nc.tensor.matmul(
                    h2_t[0:80, pp * lp:(pp + 1) * lp],
                    vws[:, rr * 80:rr * 80 + 80],
                    kxs[:, rr * lp:(rr + 1) * lp],
                    start=True, stop=True,
                )
            t_t = tp.tile([80, GP * 128], BF16, tag="t", name=f"t_{g}")
            nc.scalar.activation(
                t_t[:, 0:gc], h2_t[0:80, 0:gc], Tanh, scale=0.5)

            if g % 2 == 0:
                lg_t = lgp.tile([98, 512], F32, tag="lg", name=f"lg_{g // 2}")
            for j in range(2):
                ss = 2 * (g % 2) + j
                nc.tensor.matmul(
                    lg_t[32 * ss:32 * ss + 2, 0:4 * lp],
                    wf_t[:], t_t[:, j * 4 * lp:(j + 1) * 4 * lp],
                    start=True, stop=True, tile_position=(0, 32 * ss),
                )
            if g % 2 == 1:
                gq = (g // 2) % 4
                if gq == 0:
                    st_t = stp.tile([98, 4 * 512], BF16, tag="st", name=f"st{qq}")
                if gq == 1:
                    nc.vector.tensor_copy(
                        st_t[:, 4 * lp * gq:4 * lp * (gq + 1)],
                        lg_t[:, 0:4 * lp])
                else:
                    nc.scalar.activation(
                        st_t[:, 4 * lp * gq:4 * lp * (gq + 1)],
                        lg_t[:, 0:4 * lp], Copy)

                if gq == 3:
                    lm_t = lmp.tile([128, 128], BF16, tag="lm", name=f"lm{qq}")
                    for ss in range(4):
                        deng = nc.scalar if (qq == 3 and ss % 2 == 0) \
                            else nc.gpsimd
                        deng.dma_start(
                            lm_t[32 * ss:32 * ss + 32, 0:lp],
                            st_t[32 * ss:32 * ss + 2, 0:16 * lp])
                    phase_b_pre(qq, lm_t)
        phase_b(3)

    if not nc.is_finalized():
        nc.finalize()
    return nc


def host_prep(q, k, v, mask, W1, b1, W2, b2, Wf, bf):
    """Fold weights per batch, compact masked tokens, build device layouts."""
    q2 = q[:, 0, :].astype(np.float32)
    W1 = W1.astype(np.float32); W2 = W2.astype(np.float32)
    P = (W1[64:128] - W1[128:192]) @ W2
    Q = W1[192:256] @ W2
    A2 = (W1[0:64] + W1[128:192]) @ W2
    c0 = b1.astype(np.float32) @ W2 + b2.astype(np.float32)
    r = q2 @ A2 + c0
    V = P[None] + q2[:, :, None] * Q[None]                  # [Bf,64,40]
    G = np.einsum('beh,bei->bhi', V, V)
    y = np.linalg.solve(G, r[:, :, None])
    s = np.einsum('beh,bhx->be', V, y)

    m = mask[:, :, 0]
    order = np.argsort(m, axis=1, kind='stable')[:, :128]
    nvalid = (~m).sum(1)
    validc = np.arange(128)[None, :] < nvalid[:, None]
    kc = np.take_along_axis(k.astype(np.float32), order[:, :, None], 1)
    vc = np.take_along_axis(v.astype(np.float32), order[:, :, None], 1)
    kc = np.where(validc[..., None], kc + s[:, None, :], 0.0)
    vc = np.where(validc[..., None], vc, 0.0)
    # ones-row rides along v so the softmax denominator comes out of the
    # same fold+reduce chain (pads contribute 0)
    vce = np.concatenate([vc, validc[:, :, None].astype(np.float32)], axis=2)

    # pair rq -> within-quarter positions of its A/B batches
    rqi = np.arange(64)
    tA = 32 * ((rqi % 16) // 4) + 4 * (rqi // 16) + rqi % 4
    tB = tA + 16

    in_maps = []
    invs = []
    for c in range(N_CORES):
        sl = slice(c * B, (c + 1) * B)
        ncore = nvalid[sl]
        ordr = np.argsort(-ncore, kind='stable')   # desc by unmasked count
        invs.append(ordr)
        kcc, vcc, Vc = kc[sl], vce[sl], V[sl]
        kx = np.zeros((128, NKX), np.float32)
        vw3 = np.zeros((NPAIR, 128, 80), np.float32)
        imap = {}
        for qq in range(4):
            lp = LPS[qq]
            ranks = ordr[128 * qq:128 * (qq + 1)]
            assert ncore[ranks].max() <= lp, \
                f"core {c} q{qq}: {ncore[ranks].max()} > {lp}"
            Ab = ranks[tA]
            Bb = ranks[tB]
            kx[0:64, KB[qq]:KB[qq + 1]] = \
                kcc[Ab][:, :lp].transpose(2, 0, 1).reshape(64, -1)
            kx[64:128, KB[qq]:KB[qq + 1]] = \
                kcc[Bb][:, :lp].transpose(2, 0, 1).reshape(64, -1)
            vw3[64 * qq:64 * (qq + 1), 0:64, 0:40] = Vc[Ab]
            vw3[64 * qq:64 * (qq + 1), 64:128, 40:80] = Vc[Bb]
            imap[f"vt{qq}"] = np.ascontiguousarray(
                vcc[ranks][:, :lp].transpose(0, 2, 1)
            ).reshape(128, E2 * lp).astype(nbf16)
        vw = vw3.transpose(1, 0, 2).reshape(128, NPAIR * 80)
        wf2 = np.zeros((80, 2), np.float32)
        wf2[0:40, 0] = 0.5 * Wf[:, 0]
        wf2[40:80, 1] = 0.5 * Wf[:, 0]
        imap["kx"] = np.ascontiguousarray(kx).astype(nfp8)
        imap["vw"] = np.ascontiguousarray(vw).astype(nfp8)
        imap["wf2"] = wf2.astype(nbf16)
        in_maps.append(imap)
    return in_maps, invs


_CACHE = {}


def run_on_device(in_maps, trace=False):
    if "nc" not in _CACHE:
        _CACHE["nc"] = build_nc()
    nc = _CACHE["nc"]
    res = run_bass_kernel_spmd(nc, in_maps, core_ids=list(range(N_CORES)),
                               trace=trace)
    return res


def gather_out(res, invs):
    outs = []
    for c in range(N_CORES):
        oc = np.empty((B, E), np.float32)
        oc[invs[c]] = res.results[c]["out"]
        outs.append(oc)
    return np.concatenate(outs, axis=0)


def kernel(q, k, v, mask, W1, b1, W2, b2, Wf, bf):
    in_maps, invs = host_prep(q, k, v, mask, W1, b1, W2, b2, Wf, bf)
    res = run_on_device(in_maps)
    return gather_out(res, invs).astype(np.float32)
